# revision 2
# baseline (speedup 1.0000x reference)
"""2-layer GAT on 8 Trainium2 NeuronCores (Bass/Tile) — v3.

Changes vs v2 (lane-major baseline):
  * Gather elements trimmed to the useful payload: layer-1 elems are
    [h(128) | A8(8) | C8(8)] f16 = 288B from the 512B-pitch pair-row table
    (elem_size need not be a 256B multiple for non-transpose gathers — only
    elem_step is; bass's conservative assert is patched). Layer-2 elems are
    [z(16) | A1 | C1] f16 = 36B from 128B-pitch pair rows.
  * GMAX=896: 16*(896/16+1)=912 descriptors fit the 1024-desc SWDGE ring,
    so calls pipeline instead of serializing (1024-idx calls need 1040).
  * A8=exp(as-M), C8=exp(0.2as-M), G8=exp(-0.8ad) (and layer-2 A1/C1/G1,
    self-loop weights) precomputed on HOST between NEFFs: no per-edge exps
    on device; w = max(A8, C8*G8) is one small TT per (block, section) plus
    one group-wide max.
  * Self-loops are dense chunks heading each block's odd section (co forced
    odd, ce even, so segment sums are uniform 2-chunk matmuls): h arrives by
    plain DMA, w_self by DMA into the rhs weight column — zero gather
    descriptors and zero extra compute instructions for self-loops.
  * Groups of 4 blocks share gather calls and group-wide DVE ops; epilogues
    (reciprocal on ACT, ELU via Relu/Exp+min, output matmuls) are batched
    per group.
"""

import sys

sys.path.insert(0, "/opt/trn_rl_repo")

import numpy as np

import concourse.bacc as bacc
import concourse.mybir as mybir
import concourse.tile as tile


def _patch_dma_gather():
    """Relax bass's elem_size%256 assert for non-transpose dma_gather.

    The HW decode (dma_gather.hpp) only requires 256B-divisibility for
    transpose=True; the non-transpose path builds one descriptor per index of
    elem_size bytes at stride elem_step (which IS encoded in 256B units, so
    elem_step must stay a multiple of 256B). Verified correct on HW for 36B
    and 160B elements.
    """
    import inspect
    import textwrap

    import concourse.bass as bass

    src = textwrap.dedent(inspect.getsource(bass.BassGpSimd.dma_gather))
    old = """    assert (
        elem_size_bytes > 0 and elem_size_bytes % 256 == 0
    )  # transpose restriction"""
    new = """    assert elem_size_bytes > 0
    if transpose:
        assert elem_size_bytes % 256 == 0  # transpose restriction"""
    if old in src:
        src = src.replace(old, new)
        ns = dict(bass.__dict__)
        exec(compile(src, "<patched dma_gather>", "exec"), ns)
        bass.BassGpSimd.dma_gather = ns["dma_gather"]


_patch_dma_gather()

# ---------------- problem constants (hardcoded per task contract) -------------
N = 50000
F_IN = 128
HID = 16
HEADS = 8
CLASSES = 16
NEG = 0.2
M_SHIFT = 4.0

N_CORES = 8
P = 128
BLOCKS = 49
NODES_PER_CORE = BLOCKS * P                # 6272
N_PAD = N_CORES * NODES_PER_CORE           # 50176
TROWS = 65536
PAD_EVEN = N_PAD
PAD_ODD = N_PAD + 1
COPY0 = N_PAD + 2

BPG = 4                                    # blocks per group
GMAX = 896                                 # indices per dma_gather call
NQ = 4
SCRATCH = 16384

FP8H = False                               # fp8 h: passes accuracy but slower wh on DVE
ROW1 = 144                                 # NEFF-A output row (f16): h|as|ad
ROW1B = 160 if FP8H else 288               # layer-1 elem bytes: h|A8|C8
PITCH1B = 512                              # layer-1 table row pitch (bytes)
ROW2 = 18                                  # layer-2 elem (f16): z|A1|C1
PITCH2 = 64                                # layer-2 table row pitch (f16)

F16 = mybir.dt.float16
F32 = mybir.dt.float32
I16 = mybir.dt.int16
U8 = mybir.dt.uint8
F8 = mybir.dt.float8e4

_cache = {}
_last_cfg = None
_last_inputs = None


# ---------------------------- host preprocessing -----------------------------

def _wrap16(a):
    n = a.shape[0]
    assert n % 16 == 0
    w = a.reshape(n // 16, 16).T.astype(np.int16)
    return np.tile(w, (8, 1))


def _cumcount(keys):
    n = len(keys)
    if n == 0:
        return np.zeros(0, np.int64)
    starts = np.r_[0, np.flatnonzero(np.diff(keys)) + 1]
    lens = np.diff(np.r_[starts, n])
    return np.arange(n) - np.repeat(starts, lens)


def host_prep(edge_index):
    src = edge_index[0].astype(np.int64)
    dst = edge_index[1].astype(np.int64)
    deg = np.bincount(dst, minlength=N_PAD)
    outdeg = np.bincount(src, minlength=N_PAD)

    NCOPY = TROWS - N_PAD - 2
    HALF = NCOPY // 2
    ctop = np.argsort(-outdeg, kind="stable")[:NCOPY]
    is_copy = np.zeros(N_PAD, bool)
    is_copy[ctop] = True
    strict = ~is_copy[src]
    s_src = src[strict]
    s_dst = dst[strict]

    par = (np.arange(N_PAD) & 1).astype(np.int64)
    rng_mask = ((np.arange(N_PAD) * 2654435761) % 100) < 50
    for it in range(20):
        sgn = 1 - 2 * par
        imb = np.bincount(s_dst, weights=sgn[s_src], minlength=N_PAD)
        grad = np.bincount(s_src, weights=imb[s_dst], minlength=N_PAD)
        flip = (sgn * grad) > 1.5
        flip &= ~is_copy
        if it < 14:
            flip &= rng_mask if (it % 2 == 0) else ~rng_mask
        par[flip] ^= 1

    NC_EVEN = N_PAD // 2 - HALF
    nonc = np.flatnonzero(~is_copy)
    nce = int((par[nonc] == 0).sum())
    if nce != NC_EVEN:
        want = 0 if nce < NC_EVEN else 1
        cand = nonc[par[nonc] != want]
        sgn = 1 - 2 * par
        imb = np.bincount(s_dst, weights=sgn[s_src], minlength=N_PAD)
        grad = np.bincount(s_src, weights=imb[s_dst], minlength=N_PAD)
        cost = -(sgn * grad)
        take = cand[np.argsort(cost[cand], kind="stable")[:abs(nce - NC_EVEN)]]
        par[take] ^= 1
    cop = np.flatnonzero(is_copy)
    ce_count = int((par[cop] == 0).sum())
    if ce_count != HALF:
        want = 0 if ce_count < HALF else 1
        cand = cop[par[cop] != want]
        par[cand[:abs(ce_count - HALF)]] ^= 1

    copy_row = np.full(N_PAD, -1, np.int64)
    even_cop = cop[par[cop] == 0]
    odd_cop = cop[par[cop] == 1]
    copy_row[even_cop] = np.arange(COPY0 + 1, TROWS, 2)[:len(even_cop)]
    copy_row[odd_cop] = np.arange(COPY0, TROWS, 2)[:len(odd_cop)]

    sp_par = par[src]
    flex = is_copy[src]
    nE = np.bincount(dst[(~flex) & (sp_par == 0)], minlength=N_PAD)
    nO = np.bincount(dst[(~flex) & (sp_par == 1)], minlength=N_PAD)

    core_of = np.empty(N_PAD, np.int64)
    within = np.empty(N_PAD, np.int64)
    for p in (0, 1):
        nodes_p = np.flatnonzero(par == p)
        nodes_p = nodes_p[np.argsort(-deg[nodes_p], kind="stable")]
        core_of[nodes_p] = np.arange(len(nodes_p)) % N_CORES
        within[nodes_p] = np.arange(len(nodes_p)) // N_CORES
    slot_of_node = core_of * NODES_PER_CORE + within * 2 + par
    node_of_slot = np.empty(N_PAD, np.int64)
    node_of_slot[slot_of_node] = np.arange(N_PAD)

    lane_node = np.empty((N_CORES, BLOCKS, P), np.int64)
    for k in range(N_CORES):
        nodes_k = np.flatnonzero(core_of == k)
        key = deg[nodes_k] * 100000 + nE[nodes_k]
        nodes_k = nodes_k[np.argsort(-key, kind="stable")]
        lane_node[k] = nodes_k.reshape(BLOCKS, P)
    blk_of_node = np.empty(N_PAD, np.int64)
    lane_of_node = np.empty(N_PAD, np.int64)
    blk_of_node[lane_node.reshape(-1)] = np.tile(
        np.repeat(np.arange(BLOCKS), P), N_CORES)
    lane_of_node[lane_node.reshape(-1)] = np.tile(np.arange(P), N_CORES * BLOCKS)

    # per-block caps: cE even, cO odd (self chunk heads the odd section)
    maxD = np.zeros(BLOCKS, np.int64)
    maxNE = np.zeros(BLOCKS, np.int64)
    maxNO = np.zeros(BLOCKS, np.int64)
    np.maximum.at(maxD, blk_of_node, deg)
    np.maximum.at(maxNE, blk_of_node, nE)
    np.maximum.at(maxNO, blk_of_node, nO)
    Cstar = np.maximum(maxD, maxNE + maxNO)
    cE = np.maximum(maxNE, Cstar - maxNO)
    cO = np.maximum(Cstar - cE, maxNO)

    d_n = deg
    lo = np.maximum(nE, d_n - cO[blk_of_node])
    hi = np.minimum(cE[blk_of_node], d_n - nO)
    aE_n = np.clip((d_n + 1) // 2, lo, hi)

    sec = sp_par.copy()
    fidx = np.flatnonzero(flex)
    fperm = fidx[np.argsort(dst[fidx], kind="stable")]
    cum = _cumcount(dst[fperm])
    sec[fperm] = np.where(cum < (aE_n - nE)[dst[fperm]], 0, 1)

    key = dst * 2 + sec
    eperm = np.argsort(key, kind="stable")
    cc = _cumcount(key[eperm])
    chunk = np.empty(len(dst), np.int64)
    chunk[eperm] = cc

    kk = core_of[dst]
    bb = blk_of_node[dst]
    jj = lane_of_node[dst]
    ss = slot_of_node[src]
    row = np.where(par[src] == sec, ss, copy_row[src])
    idx16 = row >> 1
    PADE16 = PAD_EVEN >> 1
    PADO16 = PAD_ODD >> 1

    grp_blocks = [list(range(g, min(g + BPG, BLOCKS)))
                  for g in range(0, BLOCKS, BPG)]
    groups = []
    si_cols = 0
    grp_of_block = np.empty(BLOCKS, np.int64)
    evoff = np.empty(BLOCKS, np.int64)
    odoff = np.empty(BLOCKS, np.int64)
    for gi, blks in enumerate(grp_blocks):
        ce = [int(cE[b]) for b in blks]
        co = [int(cO[b]) for b in blks]
        o = 0
        for i, b in enumerate(blks):
            grp_of_block[b] = gi
            evoff[b] = o
            o += ce[i]
        o2 = 0
        for i, b in enumerate(blks):
            odoff[b] = o2
            o2 += co[i]
        n_ev = sum(ce) * P
        n_od = sum(co) * P
        groups.append(dict(blocks=blks, ce=ce, co=co, col0=si_cols,
                           n_ev=n_ev, n_od=n_od))
        si_cols += (n_ev + n_od) // 16

    flat_ev = [np.tile(np.full(g["n_ev"], PADE16, np.int16)[None], (N_CORES, 1))
               for g in groups]
    flat_od = [np.tile(np.full(g["n_od"], PADO16, np.int16)[None], (N_CORES, 1))
               for g in groups]

    gii = grp_of_block[bb]
    pos = np.where(sec == 0,
                   (evoff[bb] + chunk) * P + jj,
                   (odoff[bb] + chunk) * P + jj)
    for gi in range(len(groups)):
        m0 = (gii == gi) & (sec == 0)
        m1 = (gii == gi) & (sec == 1)
        flat_ev[gi][kk[m0], pos[m0]] = idx16[m0].astype(np.int16)
        flat_od[gi][kk[m1], pos[m1]] = idx16[m1].astype(np.int16)

    si_all = np.empty((N_CORES, 128, si_cols), np.int16)
    for k in range(N_CORES):
        parts = []
        for gi in range(len(groups)):
            parts.append(_wrap16(flat_ev[gi][k]))
            parts.append(_wrap16(flat_od[gi][k]))
        si_all[k] = np.concatenate(parts, axis=1)

    lane_slot = slot_of_node[lane_node]
    return dict(slot_of_node=slot_of_node, node_of_slot=node_of_slot,
                copy_rows=copy_row[cop], copy_slots=slot_of_node[cop],
                lane_node=lane_node, lane_slot=lane_slot,
                groups=groups, si_cols=si_cols, si_all=si_all)


# ------------------------------- NEFF builders -------------------------------

def build_neff_a(reps=1):
    TPB = 3
    nc = bacc.Bacc()
    xT = nc.dram_tensor("xT", [P, NODES_PER_CORE], F16, kind="ExternalInput")
    w1e = nc.dram_tensor("w1e", [P, ROW1], F16, kind="ExternalInput")
    g_out = nc.dram_tensor("g_out", [BLOCKS, P, ROW1], F16, kind="ExternalOutput")
    with tile.TileContext(nc) as tc:
        with tc.tile_pool(name="sbuf", bufs=4) as pool, \
             tc.tile_pool(name="psum", bufs=4, space="PSUM") as pp:
            w1t = pool.tile([P, ROW1], F16)
            nc.sync.dma_start(w1t[:], w1e[:])

            def body():
                xt = pool.tile([P, NODES_PER_CORE], F16, tag="xt", name="xt")
                nc.sync.dma_start(xt[:], xT[:])
                for t0 in range(0, BLOCKS, TPB):
                    nb = min(TPB, BLOCKS - t0)
                    ps = pp.tile([P, nb, ROW1], F32, tag="ps", space="PSUM")
                    for i in range(nb):
                        t = t0 + i
                        nc.tensor.matmul(out=ps[:, i, :],
                                         lhsT=xt[:, t * P:(t + 1) * P],
                                         rhs=w1t[:], start=True, stop=True)
                    gt = pool.tile([P, nb, ROW1], F16, tag="gt", name="gt")
                    nc.scalar.activation(gt[:], ps[:],
                                         mybir.ActivationFunctionType.Copy)
                    nc.sync.dma_start(
                        g_out[t0:t0 + nb].rearrange("c p f -> p c f"), gt[:])

            if reps == 1:
                body()
            else:
                with tc.For_i(0, reps, 1):
                    body()
    nc.finalize()
    return nc


def build_neff_b(cfg, reps=1, b1_any=False):
    nc = bacc.Bacc(num_swdge_queues=NQ, dynamic_dma_scratch_size=SCRATCH)
    groups = cfg["groups"]
    si_cols = cfg["si_cols"]
    NG = len(groups)
    g_d = nc.dram_tensor("g", [TROWS // 2, 2, PITCH1B], U8, kind="ExternalInput")
    si_d = nc.dram_tensor("si", [128, si_cols], I16, kind="ExternalInput")
    own_d = nc.dram_tensor("own", [BLOCKS, P, P * (1 if FP8H else 2)], U8,
                           kind="ExternalInput")
    wself_d = nc.dram_tensor("wself", [BLOCKS, P, HEADS], F16, kind="ExternalInput")
    g8_d = nc.dram_tensor("g8t", [P, BLOCKS * HEADS], F16, kind="ExternalInput")
    identf_d = nc.dram_tensor("identf", [P, P], F16, kind="ExternalInput")
    ident32_d = nc.dram_tensor("ident32", [P, P], F32, kind="ExternalInput")
    b1r_d = nc.dram_tensor("b1r", [P, P], F32, kind="ExternalInput")
    w2e_d = nc.dram_tensor("w2e", [P, ROW2], F16, kind="ExternalInput")
    g2_out = nc.dram_tensor("g2_out", [NG, P, BPG, ROW2], F16, kind="ExternalOutput")

    qctr = [0]

    def qrr():
        qctr[0] = (qctr[0] + 1) % NQ
        return qctr[0]

    Exp = mybir.ActivationFunctionType.Exp
    Relu = mybir.ActivationFunctionType.Relu
    Copy = mybir.ActivationFunctionType.Copy
    Recip = mybir.ActivationFunctionType.Reciprocal

    with tile.TileContext(nc) as tc:
        with tc.tile_pool(name="sbuf", bufs=2) as pool, \
             tc.tile_pool(name="sbuf_s", bufs=3) as spool, \
             tc.tile_pool(name="sbuf_c", bufs=1) as cpool, \
             tc.tile_pool(name="psum", bufs=2, space="PSUM") as pp:
            identf = cpool.tile([P, P], F16)
            nc.sync.dma_start(identf[:], identf_d[:])
            ident32 = cpool.tile([P, P], F32)
            nc.sync.dma_start(ident32[:], ident32_d[:])
            b1r = cpool.tile([P, P], F32)
            nc.sync.dma_start(b1r[:], b1r_d[:])
            w2e = cpool.tile([P, ROW2], F16)
            nc.sync.dma_start(w2e[:], w2e_d[:])
            g8t = cpool.tile([P, BLOCKS * HEADS], F16)
            nc.sync.dma_start(g8t[:], g8_d[:])
            si = cpool.tile([128, si_cols], I16)
            nc.sync.dma_start(si[:], si_d[:])

            def body():
                for gi, info in enumerate(groups):
                    blks = info["blocks"]
                    G = len(blks)
                    ce, co = info["ce"], info["co"]
                    nEv = info["n_ev"] // P
                    nOd = info["n_od"] // P
                    ng = nEv + nOd
                    zc = ng                          # shared zero chunk
                    ch_g = ng + 1 + G                # + zero + self chunks
                    X = pool.tile([P, ch_g, ROW1B], U8, tag="X")
                    col0 = info["col0"]
                    # even + odd sections: group-contiguous runs of calls
                    for base, nsec, w in ((0, info["n_ev"], 0),
                                          (info["n_ev"], info["n_od"], 1)):
                        for b0 in range(0, nsec, GMAX):
                            n = min(GMAX, nsec - b0)
                            nc.gpsimd.dma_gather(
                                out_ap=X[:, (base + b0) // P:(base + b0 + n) // P, :],
                                in_ap=g_d[:, w, 0:ROW1B],
                                idxs_ap=si[:, col0 + (base + b0) // 16:
                                           col0 + (base + b0 + n) // 16],
                                num_idxs=n, num_idxs_reg=n, elem_size=ROW1B,
                                elem_step=2 * PITCH1B, queue_num=qrr())
                    HB = P if FP8H else 2 * P
                    for i in range(G):
                        nc.sync.dma_start(X[:, zc + 1 + i, 0:HB], own_d[blks[i]])

                    rhs = pool.tile([P, ch_g, 136], F16, tag="rhs")
                    cg = pool.tile([P, ch_g, HEADS], F16, tag="cg")
                    evo = 0
                    odo = nEv
                    for i, b in enumerate(blks):
                        g8b = g8t[:, b * HEADS:(b + 1) * HEADS]
                        for (xo, ln) in ((evo, ce[i]), (odo, co[i])):
                            if ln:
                                nc.vector.tensor_tensor(
                                    out=cg[:, xo:xo + ln, :],
                                    in0=X[:, xo:xo + ln, HB + 16:HB + 32].bitcast(F16),
                                    in1=g8b[:, None, :].to_broadcast([P, ln, HEADS]),
                                    op=mybir.AluOpType.mult)
                        evo += ce[i]
                        odo += co[i]
                    # w = max(A8, cg) over everything (tail slots get garbage;
                    # self overwritten by wself DMAs, zero chunk by the memset)
                    nc.vector.tensor_tensor(
                        out=rhs[:, :, 128:136], in0=cg[:],
                        in1=X[:, :, HB:HB + 16].bitcast(F16),
                        op=mybir.AluOpType.max)
                    for i, b in enumerate(blks):
                        nc.sync.dma_start(rhs[:, zc + 1 + i, 128:136], wself_d[b])
                    # wh = h * w for all chunks
                    nc.vector.tensor_tensor(
                        out=rhs[:, :, 0:128].rearrange("p c (h k) -> p c h k", k=HID),
                        in0=X[:, :, 0:HB].bitcast(F8 if FP8H else F16).rearrange(
                            "p c (h k) -> p c h k", k=HID),
                        in1=rhs[:, :, 128:136, None].to_broadcast(
                            [P, ch_g, HEADS, HID]),
                        op=mybir.AluOpType.mult)
                    nc.vector.memset(rhs[:, zc, :], 0.0)

                    stage = spool.tile([P, BPG, 136], F32, tag="stage")
                    evo = 0
                    odo = nEv
                    for i, b in enumerate(blks):
                        acc2 = pp.tile([P, 272], F32, tag="acc", space="PSUM")
                        frags = []
                        for (xo, ln) in ((evo, ce[i]), (odo, co[i])):
                            for j in range(ln // 2):
                                frags.append(rhs[:, xo + 2 * j:xo + 2 * j + 2, :]
                                             .rearrange("p c f -> p (c f)"))
                            if ln & 1:
                                e = xo + ln - 1
                                frags.append(rhs[:, e:zc + 1:zc - e, :])
                        frags.append(rhs[:, zc:zc + i + 2:i + 1, :])
                        for fi, fr in enumerate(frags):
                            nc.tensor.matmul(
                                out=acc2[:], lhsT=identf[:], rhs=fr,
                                start=(fi == 0), stop=(fi == len(frags) - 1))
                        nc.scalar.activation(stage[:, i, :], acc2[:, 0:136],
                                             Copy)
                        nc.vector.tensor_tensor(
                            out=stage[:, i, :], in0=stage[:, i, :],
                            in1=acc2[:, 136:272], op=mybir.AluOpType.add)
                        evo += ce[i]
                        odo += co[i]

                    rec = spool.tile([P, BPG, HEADS], F32, tag="rec")
                    nc.vector.reciprocal(rec[:, 0:G, :], stage[:, 0:G, 128:136])
                    o1 = spool.tile([P, BPG, P], F16, tag="o1")
                    nc.vector.tensor_tensor(
                        out=o1[:, 0:G].rearrange("p g (h k) -> p g h k", k=HID),
                        in0=stage[:, 0:G, 0:128].rearrange("p g (h k) -> p g h k", k=HID),
                        in1=rec[:, 0:G, :, None].to_broadcast([P, G, HEADS, HID]),
                        op=mybir.AluOpType.mult)
                    if b1_any:
                        nc.vector.tensor_tensor(
                            out=o1[:, 0:G], in0=o1[:, 0:G],
                            in1=b1r[:, None, :].to_broadcast([P, G, P]),
                            op=mybir.AluOpType.add)
                    rx = spool.tile([P, BPG, P], F16, tag="rx")
                    nc.scalar.activation(rx[:, 0:G], o1[:, 0:G], Relu)
                    ev_ = spool.tile([P, BPG, P], F16, tag="ev")
                    nc.scalar.activation(ev_[:, 0:G], o1[:, 0:G], Exp)
                    nc.vector.tensor_scalar(
                        out=ev_[:, 0:G], in0=ev_[:, 0:G], scalar1=-1.0, scalar2=0.0,
                        op0=mybir.AluOpType.add, op1=mybir.AluOpType.min)
                    elu = spool.tile([P, BPG, P], F32, tag="elu")
                    nc.vector.tensor_tensor(out=elu[:, 0:G], in0=ev_[:, 0:G],
                                            in1=rx[:, 0:G], op=mybir.AluOpType.add)

                    g2s = spool.tile([P, BPG, ROW2], F16, tag="g2s")
                    for i in range(G):
                        eTp = pp.tile([P, P], F32, tag="eTp", space="PSUM")
                        nc.tensor.transpose(out=eTp[:], in_=elu[:, i, :],
                                            identity=ident32[:])
                        eT = spool.tile([P, P], F16, tag="eT")
                        nc.scalar.activation(eT[:], eTp[:], Copy)
                        g2p = pp.tile([P, ROW2], F32, tag="g2p", space="PSUM")
                        nc.tensor.matmul(out=g2p[:], lhsT=eT[:], rhs=w2e[:],
                                         start=True, stop=True)
                        nc.scalar.activation(g2s[:, i, :], g2p[:], Copy)
                    nc.sync.dma_start(g2_out[gi, :, 0:G, :], g2s[:, 0:G])

            if reps == 1:
                body()
            else:
                with tc.For_i(0, reps, 1):
                    body()
    nc.finalize()
    return nc


def build_neff_c(cfg, reps=1, b2_any=False):
    nc = bacc.Bacc(num_swdge_queues=NQ, dynamic_dma_scratch_size=SCRATCH)
    groups = cfg["groups"]
    si_cols = cfg["si_cols"]
    NG = len(groups)
    g_d = nc.dram_tensor("g2", [TROWS // 2, 2, PITCH2], F16, kind="ExternalInput")
    si_d = nc.dram_tensor("si", [128, si_cols], I16, kind="ExternalInput")
    own_d = nc.dram_tensor("own2", [BLOCKS, P, CLASSES], F16, kind="ExternalInput")
    wself_d = nc.dram_tensor("wself2", [BLOCKS, P, 1], F16, kind="ExternalInput")
    g1_d = nc.dram_tensor("g1t", [P, BLOCKS], F16, kind="ExternalInput")
    identf_d = nc.dram_tensor("identf", [P, P], F16, kind="ExternalInput")
    b2r_d = nc.dram_tensor("b2r", [P, CLASSES], F32, kind="ExternalInput")
    out_d = nc.dram_tensor("out2", [NG, P, BPG, CLASSES], F32, kind="ExternalOutput")

    qctr = [0]

    def qrr():
        qctr[0] = (qctr[0] + 1) % NQ
        return qctr[0]

    Recip = mybir.ActivationFunctionType.Reciprocal

    with tile.TileContext(nc) as tc:
        with tc.tile_pool(name="sbuf", bufs=2) as pool, \
             tc.tile_pool(name="sbuf_s", bufs=3) as spool, \
             tc.tile_pool(name="sbuf_c", bufs=1) as cpool, \
             tc.tile_pool(name="psum", bufs=2, space="PSUM") as pp:
            identf = cpool.tile([P, P], F16)
            nc.sync.dma_start(identf[:], identf_d[:])
            b2r = cpool.tile([P, CLASSES], F32)
            nc.sync.dma_start(b2r[:], b2r_d[:])
            g1t = cpool.tile([P, BLOCKS], F16)
            nc.sync.dma_start(g1t[:], g1_d[:])
            si = cpool.tile([128, si_cols], I16)
            nc.sync.dma_start(si[:], si_d[:])

            def body():
                for gi, info in enumerate(groups):
                    blks = info["blocks"]
                    G = len(blks)
                    ce, co = info["ce"], info["co"]
                    nEv = info["n_ev"] // P
                    nOd = info["n_od"] // P
                    ng = nEv + nOd
                    zc = ng
                    ch_g = ng + 1 + G
                    X = pool.tile([P, ch_g, ROW2], F16, tag="X")
                    col0 = info["col0"]
                    for base, nsec, w in ((0, info["n_ev"], 0),
                                          (info["n_ev"], info["n_od"], 1)):
                        for b0 in range(0, nsec, GMAX):
                            n = min(GMAX, nsec - b0)
                            nc.gpsimd.dma_gather(
                                out_ap=X[:, (base + b0) // P:(base + b0 + n) // P, :],
                                in_ap=g_d[:, w, 0:ROW2],
                                idxs_ap=si[:, col0 + (base + b0) // 16:
                                           col0 + (base + b0 + n) // 16],
                                num_idxs=n, num_idxs_reg=n, elem_size=ROW2,
                                elem_step=2 * PITCH2, queue_num=qrr())
                    for i in range(G):
                        nc.sync.dma_start(X[:, zc + 1 + i, 0:CLASSES],
                                          own_d[blks[i]])

                    rhs = pool.tile([P, ch_g, 17], F16, tag="rhs")
                    for i, b in enumerate(blks):
                        nc.sync.dma_start(rhs[:, zc + 1 + i, 16:17], wself_d[b])
                    evo = 0
                    odo = nEv
                    for i, b in enumerate(blks):
                        for (xo, ln) in ((evo, ce[i]), (odo, co[i])):
                            if ln:
                                nc.vector.scalar_tensor_tensor(
                                    out=rhs[:, xo:xo + ln, 16:17],
                                    in0=X[:, xo:xo + ln, 17:18],
                                    scalar=g1t[:, b:b + 1],
                                    in1=X[:, xo:xo + ln, 16:17],
                                    op0=mybir.AluOpType.mult,
                                    op1=mybir.AluOpType.max)
                        evo += ce[i]
                        odo += co[i]
                    nc.vector.tensor_tensor(
                        out=rhs[:, :, 0:16], in0=X[:, :, 0:16],
                        in1=rhs[:, :, 16:17].to_broadcast([P, ch_g, 16]),
                        op=mybir.AluOpType.mult)
                    nc.vector.memset(rhs[:, zc, :], 0.0)

                    stage = spool.tile([P, BPG, 17], F32, tag="stage")
                    evo = 0
                    odo = nEv
                    for i, b in enumerate(blks):
                        acc2 = pp.tile([P, 34], F32, tag="acc", space="PSUM")
                        frags = []
                        for (xo, ln) in ((evo, ce[i]), (odo, co[i])):
                            for j in range(ln // 2):
                                frags.append(rhs[:, xo + 2 * j:xo + 2 * j + 2, :]
                                             .rearrange("p c f -> p (c f)"))
                            if ln & 1:
                                e = xo + ln - 1
                                frags.append(rhs[:, e:zc + 1:zc - e, :])
                        frags.append(rhs[:, zc:zc + i + 2:i + 1, :])
                        for fi, fr in enumerate(frags):
                            nc.tensor.matmul(
                                out=acc2[:], lhsT=identf[:], rhs=fr,
                                start=(fi == 0), stop=(fi == len(frags) - 1))
                        nc.scalar.activation(stage[:, i, :], acc2[:, 0:17],
                                             mybir.ActivationFunctionType.Copy)
                        nc.vector.tensor_tensor(
                            out=stage[:, i, :], in0=stage[:, i, :],
                            in1=acc2[:, 17:34], op=mybir.AluOpType.add)
                        evo += ce[i]
                        odo += co[i]

                    rec = spool.tile([P, BPG, 1], F32, tag="rec")
                    nc.vector.reciprocal(rec[:, 0:G], stage[:, 0:G, 16:17])
                    o2 = spool.tile([P, BPG, CLASSES], F32, tag="o2")
                    nc.vector.tensor_tensor(
                        out=o2[:, 0:G], in0=stage[:, 0:G, 0:16],
                        in1=rec[:, 0:G].to_broadcast([P, G, CLASSES]),
                        op=mybir.AluOpType.mult)
                    if b2_any:
                        nc.vector.tensor_tensor(
                            out=o2[:, 0:G], in0=o2[:, 0:G],
                            in1=b2r[:, None, :].to_broadcast([P, G, CLASSES]),
                            op=mybir.AluOpType.add)
                    nc.sync.dma_start(out_d[gi, :, 0:G, :], o2[:, 0:G])

            if reps == 1:
                body()
            else:
                with tc.For_i(0, reps, 1):
                    body()
    nc.finalize()
    return nc


# ------------------------------ runner plumbing ------------------------------

def make_runner(nc, n_cores=N_CORES):
    import jax
    from jax.sharding import Mesh, PartitionSpec
    from jax.experimental.shard_map import shard_map
    from concourse.bass2jax import _bass_exec_p, install_neuronx_cc_hook, partition_id_tensor

    install_neuronx_cc_hook()
    partition_name = nc.partition_id_tensor.name if nc.partition_id_tensor else None
    in_names, out_names, out_avals = [], [], []
    for alloc in nc.m.functions[0].allocations:
        if not isinstance(alloc, mybir.MemoryLocationSet):
            continue
        name = alloc.memorylocations[0].name
        if alloc.kind == "ExternalInput":
            if name != partition_name:
                in_names.append(name)
        elif alloc.kind == "ExternalOutput":
            out_names.append(name)
            out_avals.append(jax.core.ShapedArray(tuple(alloc.tensor_shape),
                                                  mybir.dt.np(alloc.dtype)))
    n_params = len(in_names)
    all_names = in_names + out_names + ([partition_name] if partition_name else [])

    def _body(*args):
        operands = list(args)
        if partition_name is not None:
            operands.append(partition_id_tensor())
        return tuple(_bass_exec_p.bind(
            *operands, out_avals=tuple(out_avals), in_names=tuple(all_names),
            out_names=tuple(out_names), lowering_input_output_aliases=(),
            sim_require_finite=False, sim_require_nnan=False, nc=nc))

    devices = jax.devices()[:n_cores]
    mesh = Mesh(np.asarray(devices), ("core",))
    sharded = jax.jit(
        shard_map(_body, mesh=mesh,
                  in_specs=(PartitionSpec("core"),) * (n_params + len(out_names)),
                  out_specs=(PartitionSpec("core"),) * len(out_names),
                  check_rep=False),
        keep_unused=True)

    import jax as _jax
    from jax.sharding import NamedSharding

    _dev_args = {}

    def run(in_maps, key=None, raw=False):
        if key is not None and key in _dev_args:
            args = _dev_args[key]
        else:
            concat_in = [np.concatenate([np.asarray(m[nm]) for m in in_maps], axis=0)
                         for nm in in_names]
            concat_zero = [np.zeros((n_cores * a.shape[0], *a.shape[1:]), a.dtype)
                           for a in out_avals]
            sh = NamedSharding(mesh, PartitionSpec("core"))
            args = [_jax.device_put(a, sh) for a in concat_in + concat_zero]
            _jax.block_until_ready(args)
            if key is not None:
                _dev_args[key] = args
        outs = sharded(*args)
        _jax.block_until_ready(outs)
        if raw:
            return outs
        return [
            {nm: np.asarray(outs[i]).reshape(n_cores, *out_avals[i].shape)[c]
             for i, nm in enumerate(out_names)}
            for c in range(n_cores)
        ]

    return run


def _get_compiled(key, builder):
    if key not in _cache:
        nc = builder()
        _cache[key] = make_runner(nc)
    return _cache[key]


# --------------------------------- kernel ------------------------------------

def kernel(x, edge_index, W1, a_src1, a_dst1, b1, W2, a_src2, a_dst2, b2):
    x = np.asarray(x, np.float32)
    edge_index = np.asarray(edge_index)
    W1 = np.asarray(W1, np.float32)
    W2 = np.asarray(W2, np.float32)
    a_src1 = np.asarray(a_src1, np.float32)
    a_dst1 = np.asarray(a_dst1, np.float32)
    a_src2 = np.asarray(a_src2, np.float32)
    a_dst2 = np.asarray(a_dst2, np.float32)
    b1 = np.asarray(b1, np.float32)
    b2 = np.asarray(b2, np.float32)
    b1_any = bool(np.any(b1))
    b2_any = bool(np.any(b2))

    cfg = host_prep(edge_index)
    nos = cfg["node_of_slot"]
    lane_slot = cfg["lane_slot"]                     # [8, 49, 128] slot ids

    As = np.zeros((P, HEADS), np.float32)
    Ad = np.zeros((P, HEADS), np.float32)
    for h in range(HEADS):
        As[h * HID:(h + 1) * HID, h] = a_src1[h]
        Ad[h * HID:(h + 1) * HID, h] = a_dst1[h]
    W1ext = np.concatenate([W1, W1 @ As, W1 @ Ad], 1).astype(np.float16)
    W2ext = np.concatenate([W2, W2 @ a_src2.T, W2 @ a_dst2.T], 1).astype(np.float16)
    identf = np.eye(P, dtype=np.float16)
    ident32 = np.eye(P, dtype=np.float32)
    b1r = np.ascontiguousarray(np.broadcast_to(b1, (P, P))).astype(np.float32)
    b2r = np.ascontiguousarray(np.broadcast_to(b2, (P, CLASSES))).astype(np.float32)

    xfull = np.zeros((N_PAD, F_IN), np.float32)
    xfull[:N] = x
    xp = xfull[nos]
    xT = np.ascontiguousarray(xp.T).astype(np.float16)

    # ---- NEFF-A: g = [h | as | ad] per slot ----
    run_a = _get_compiled("A", build_neff_a)
    in_a = [{"xT": np.ascontiguousarray(xT[:, k * NODES_PER_CORE:(k + 1) * NODES_PER_CORE]),
             "w1e": W1ext} for k in range(N_CORES)]
    res_a = run_a(in_a)
    g_host = np.concatenate(
        [res_a[k]["g_out"].reshape(NODES_PER_CORE, ROW1)
         for k in range(N_CORES)], axis=0).astype(np.float32)  # [N_PAD(slot), 144]

    h_n = g_host[:, 0:128]
    as_n = g_host[:, 128:136]
    ad_n = g_host[:, 136:144]
    A8 = np.exp(as_n - M_SHIFT)
    C8 = np.exp(NEG * as_n - M_SHIFT)
    G8 = np.exp(-0.8 * ad_n)
    wself1 = np.maximum(A8, C8 * G8).astype(np.float16)

    if FP8H:
        from ml_dtypes import float8_e4m3fn
        h_bytes = h_n.astype(float8_e4m3fn).view(np.uint8)        # [N_PAD,128]
    else:
        h_bytes = h_n.astype(np.float16).view(np.uint8)           # [N_PAD,256]
    HBY = h_bytes.shape[1]
    gB = np.zeros((TROWS, PITCH1B), np.uint8)
    gB[:N_PAD, 0:HBY] = h_bytes
    gB[:N_PAD, HBY:HBY + 16] = A8.astype(np.float16).view(np.uint8)
    gB[:N_PAD, HBY + 16:HBY + 32] = C8.astype(np.float16).view(np.uint8)
    gB[cfg["copy_rows"]] = gB[cfg["copy_slots"]]
    gBp = gB.reshape(TROWS // 2, 2, PITCH1B)

    def blk_pack(arr_slots):
        """arr_slots [N_PAD, w] -> per-core [BLOCKS, P, w] f16."""
        return [np.ascontiguousarray(arr_slots[lane_slot[k]]).astype(np.float16)
                for k in range(N_CORES)]

    own_b = [np.ascontiguousarray(h_bytes[lane_slot[k]])
             for k in range(N_CORES)]
    wself_b = blk_pack(wself1)
    g8_b = []
    for k in range(N_CORES):
        g8_b.append(np.ascontiguousarray(
            G8[lane_slot[k]].transpose(1, 0, 2).reshape(P, BLOCKS * HEADS)
        ).astype(np.float16))

    run_b = _get_compiled(("B", cfg["si_cols"], b1_any),
                          lambda: build_neff_b(cfg, 1, b1_any))
    in_b = [{"g": gBp, "si": cfg["si_all"][k], "own": own_b[k],
             "wself": wself_b[k], "g8t": g8_b[k], "identf": identf,
             "ident32": ident32, "b1r": b1r, "w2e": W2ext}
            for k in range(N_CORES)]
    res_b = run_b(in_b)

    groups = cfg["groups"]
    g2_host = np.zeros((N_PAD, ROW2), np.float32)
    for k in range(N_CORES):
        gb = res_b[k]["g2_out"].astype(np.float32)   # [NG, P, BPG, 18]
        for gi, info in enumerate(groups):
            for j, b in enumerate(info["blocks"]):
                g2_host[lane_slot[k, b]] = gb[gi, :, j, :]
    bad = ~np.isfinite(g2_host).all(1)
    g2_host[bad] = 0

    z_n = g2_host[:, 0:16]
    as2_n = g2_host[:, 16:17]
    ad2_n = g2_host[:, 17:18]
    M2 = max(4.0, float(as2_n.max()))
    A1 = np.exp(as2_n - M2)
    C1 = np.exp(NEG * as2_n - M2)
    G1 = np.exp(-0.8 * ad2_n)
    wself2 = np.maximum(A1, C1 * G1).astype(np.float16)

    gC = np.zeros((TROWS, PITCH2), np.float16)
    gC[:N_PAD, 0:16] = z_n
    gC[:N_PAD, 16:17] = A1
    gC[:N_PAD, 17:18] = C1
    gC[cfg["copy_rows"]] = gC[cfg["copy_slots"]]
    gCp = gC.reshape(TROWS // 2, 2, PITCH2)

    own_c = blk_pack(z_n)
    wself_c = blk_pack(wself2)
    g1_c = []
    for k in range(N_CORES):
        g1_c.append(np.ascontiguousarray(
            G1[lane_slot[k]].transpose(1, 0, 2).reshape(P, BLOCKS)
        ).astype(np.float16))

    run_c = _get_compiled(("C", cfg["si_cols"], b2_any),
                          lambda: build_neff_c(cfg, 1, b2_any))
    in_c = [{"g2": gCp, "si": cfg["si_all"][k], "own2": own_c[k],
             "wself2": wself_c[k], "g1t": g1_c[k], "identf": identf,
             "b2r": b2r} for k in range(N_CORES)]
    res_c = run_c(in_c)

    out_slots = np.zeros((N_PAD, CLASSES), np.float32)
    for k in range(N_CORES):
        ob = res_c[k]["out2"]                        # [NG, P, BPG, 16]
        for gi, info in enumerate(groups):
            for j, b in enumerate(info["blocks"]):
                out_slots[lane_slot[k, b]] = ob[gi, :, j, :]
    out = out_slots[cfg["slot_of_node"]]
    global _last_cfg, _last_inputs
    _last_cfg = cfg
    _last_inputs = {"A": in_a, "B": in_b, "C": in_c}
    return out[:N].astype(np.float32)
